# revision 18
# baseline (speedup 1.0000x reference)
"""Trainium2 Bass kernel for the AttentionSemantic module.

Computation (per batch b):
    enc_proj = sem[b] @ W_enc + b_enc                     # [L, A]
    dec_proj = dec[b] @ W_dec + b_dec                     # [A]
    joined   = tanh(enc_proj + dec_proj)                  # [L, A]
    scores   = joined @ W_full (+ b_full, cancels in softmax)
    att_sc   = softmax(scores)                            # [L]
    att_out  = att_sc @ enc_proj                          # [A]
             = (att_sc @ sem[b]) @ W_enc + b_enc          # (sum att_sc == 1)

Sharding: data-parallel over batch B=64 across 8 cores (8 batches/core).

Device strategy (per core):
  - sem[b] cast-loaded f32->bf16 (SWDGE/gpsimd) in natural [L,E] tiles, then
    xbar-DMA-transposed (HWDGE/sync only) into semT [E(part), L(free)] bf16.
    Loads for batch b+1 are emitted before batch b's compute (SW pipeline)
    so the PE never waits on softmax-gated DMAs.
  - enc matmul: PSUM[a_chunk, l_tile] += W_enc[e_chunk, a_chunk].T @ semT,
    loop order (a_chunk, e_chunk, l_tile) to reuse the stationary operand.
  - tanh+bias fused on ACT straight out of PSUM, joined stored bf16 [A, L].
  - scores: M=1 matmuls wf.T @ joined accumulated over a_chunks.
  - softmax on one partition row [1, L] (reduce_max / Exp+accum / mul).
  - ctx = att_sc @ sem[b]: scores row bounced via DRAM (scalar-HWDGE queue),
    broadcast-loaded to 128 partitions, then DVE mult + free-dim reduce
    against semT (bf16 2x mode, chunks paired).
  - att_out = ctx.T @ W_enc + b_enc as one batched M=8 matmul at the end.
"""

import sys

if "/opt/trn_rl_repo" not in sys.path:
    sys.path.insert(0, "/opt/trn_rl_repo")

from contextlib import ExitStack

import ml_dtypes
import numpy as np

import concourse.bass as bass
import concourse.bacc as bacc
import concourse.tile as tile
from concourse import mybir
from concourse import bass_utils

P = 128
F32 = mybir.dt.float32
BF16 = mybir.dt.bfloat16
AF = mybir.ActivationFunctionType
ALU = mybir.AluOpType

# risk fallbacks (validated on hardware)
BC_ENGINE = "scalar"   # "scalar" (HWDGE) or "gpsimd" (SWDGE) broadcast load
RED_BF16 = True        # paired bf16-out reduce (2x) vs f32-out per chunk (1x)
VARIANT = "full"


def build_bass(blc, l, e, a, h, num_devices=8):
    """Build the per-core Bass program. blc = batches per core."""
    LT = l // P        # L tiles of 128 rows
    EC = e // P        # E chunks of 128
    AC = a // P        # A chunks of 128
    HC = h // P        # H chunks of 128
    NSC = 512 if l % 512 == 0 else l   # matmul moving-dim tile along L
    NLT = l // NSC

    nc = bacc.Bacc(
        "TRN2",
        target_bir_lowering=False,
        debug=False,
        enable_asserts=False,
        num_devices=num_devices,
    )

    sem = nc.dram_tensor("sem", [blc, l, e], F32, kind="ExternalInput").ap()
    dect_bf = nc.dram_tensor("dect_bf", [h, blc], BF16, kind="ExternalInput").ap()
    w_enc_bf = nc.dram_tensor("w_enc_bf", [e, a], BF16, kind="ExternalInput").ap()
    w_dec_bf = nc.dram_tensor("w_dec_bf", [h, a], BF16, kind="ExternalInput").ap()
    b_enc = nc.dram_tensor("b_enc", [a], F32, kind="ExternalInput").ap()
    b_dec = nc.dram_tensor("b_dec", [a], F32, kind="ExternalInput").ap()
    wf_bf = nc.dram_tensor("wf_bf", [a], BF16, kind="ExternalInput").ap()

    att_out = nc.dram_tensor("att_out", [blc, a], F32, kind="ExternalOutput").ap()
    att_sc = nc.dram_tensor("att_sc", [blc, l], F32, kind="ExternalOutput").ap()

    with tile.TileContext(nc) as tc, ExitStack() as ctx:
        consts = ctx.enter_context(tc.tile_pool(name="consts", bufs=1))
        natp = ctx.enter_context(tc.tile_pool(name="natp", bufs=3))
        semtp = ctx.enter_context(tc.tile_pool(name="semtp", bufs=3))
        joinp = ctx.enter_context(tc.tile_pool(name="joinp", bufs=1))
        rowp = ctx.enter_context(tc.tile_pool(name="rowp", bufs=1))
        bcp = ctx.enter_context(tc.tile_pool(name="bcp", bufs=2))
        ttrp = ctx.enter_context(tc.tile_pool(name="ttrp", bufs=1))
        dramp = ctx.enter_context(tc.tile_pool(name="dramp", bufs=2, space="DRAM"))
        ps_enc = ctx.enter_context(tc.tile_pool(name="ps_enc", bufs=6, space="PSUM"))
        ps_sc = ctx.enter_context(tc.tile_pool(name="ps_sc", bufs=2, space="PSUM"))

        # ---- constants / params ----
        w_enc_sb = consts.tile([P, EC, a], BF16)
        nc.sync.dma_start(out=w_enc_sb, in_=w_enc_bf.rearrange("(c p) a -> p c a", p=P))
        w_dec_sb = consts.tile([P, HC, a], BF16)
        nc.sync.dma_start(out=w_dec_sb, in_=w_dec_bf.rearrange("(c p) a -> p c a", p=P))
        wf_sb = consts.tile([P, AC], BF16)
        nc.sync.dma_start(out=wf_sb, in_=wf_bf.rearrange("(c p) -> p c", p=P))
        b_enc_sb = consts.tile([P, AC], F32)
        nc.sync.dma_start(out=b_enc_sb, in_=b_enc.rearrange("(c p) -> p c", p=P))
        b_dec_sb = consts.tile([P, AC], F32)
        nc.sync.dma_start(out=b_dec_sb, in_=b_dec.rearrange("(c p) -> p c", p=P))
        bsum = consts.tile([P, AC], F32)
        nc.vector.tensor_add(bsum, b_enc_sb, b_dec_sb)
        dect_sb = consts.tile([P, HC, blc], BF16)
        nc.sync.dma_start(out=dect_sb, in_=dect_bf.rearrange("(c p) b -> p c b", p=P))
        b_enc_row = consts.tile([blc, a], F32)
        nc.gpsimd.dma_start(
            out=b_enc_row,
            in_=bass.AP(tensor=b_enc.tensor, offset=b_enc.offset,
                        ap=[[0, blc], [1, a]]),
        )

        # ---- dec projection: db[a_chunk, b] = (dec @ W_dec + b_dec + b_enc).T ----
        db = consts.tile([P, AC, blc], F32)
        for j in range(AC):
            ps_dec = ps_enc.tile([P, blc], F32, tag="enc", name=f"ps_dec_{j}")
            for c in range(HC):
                nc.tensor.matmul(
                    ps_dec,
                    lhsT=w_dec_sb[:, c, j * P:(j + 1) * P],
                    rhs=dect_sb[:, c, :],
                    start=(c == 0),
                    stop=(c == HC - 1),
                )
            nc.scalar.activation(
                out=db[:, j, :], in_=ps_dec, func=AF.Identity,
                bias=bsum[:, j:j + 1], scale=1.0,
            )

        # ctx (= att_sc @ sem) for all batches, bf16, [e0, e_chunk, b]
        ctx_all = consts.tile([P, EC, blc], BF16)
        zero1 = consts.tile([1, 1], F32)
        nc.vector.memset(zero1, 0.0)
        zero1b = consts.tile([P, 1], F32)
        nc.vector.memset(zero1b, 0.0)
        se_dram = dramp.tile([blc, 1], F32, tag="se_dram", bufs=1)

        # ---- software-pipelined batch loads ----
        # semT layout [e0, t(L-tile), e_chunk, l0] — t-major so each xbar
        # transpose writes a per-partition CONTIGUOUS 2KB run (the l-major
        # layout fragmented the S2M side into 256B descriptors).
        semT_tiles = [None] * blc

        def load_batch(b):
            semT = semtp.tile([P, LT, EC, P], BF16, tag="semT", name=f"semT_{b}")
            semT_tiles[b] = semT
            KG = min(4, LT)
            for tp in range(LT // KG):
                nat = natp.tile([P, KG, e], BF16, tag="nat", name=f"nat_{b}_{tp}")
                nc.gpsimd.dma_start(
                    out=nat,
                    in_=sem[b, tp * KG * P:(tp + 1) * KG * P, :]
                    .rearrange("(k p) e -> p k e", p=P),
                )
                for k in range(KG):
                    t = tp * KG + k
                    nc.sync.dma_start(
                        out=semT[:, t, :, :],
                        in_=nat[:, k, :],
                        transpose=True,
                    )

        load_batch(0)

        # ---- main per-batch loop ----
        for b in range(blc):
            if b + 1 < blc:
                load_batch(b + 1)
            semT = semT_tiles[b]
            semT_tiles[b] = None

            # enc matmul + tanh -> joined bf16 [a0, a_chunk, l]
            joined = joinp.tile([P, AC, l], BF16, tag="joined", name=f"joined_{b}")
            for j in range(AC):
                pss = [
                    ps_enc.tile([P, NSC], F32, tag="enc", name=f"pse_{b}_{j}_{t4}")
                    for t4 in range(NLT)
                ]
                for c in range(EC):
                    for t4 in range(NLT):
                        nc.tensor.matmul(
                            pss[t4],
                            lhsT=w_enc_sb[:, c, j * P:(j + 1) * P],
                            rhs=semT[:, t4 * (NSC // P):(t4 + 1) * (NSC // P), c, :],
                            start=(c == 0),
                            stop=(c == EC - 1),
                        )
                for t4 in range(NLT):
                    nc.scalar.activation(
                        out=joined[:, j, t4 * NSC:(t4 + 1) * NSC],
                        in_=pss[t4], func=AF.Tanh,
                        bias=db[:, j, b:b + 1], scale=1.0,
                    )

            # scores -> exp(scores) straight out of PSUM, per 512-segment,
            # with per-segment partial sums. No max subtraction (|scores| <=
            # sum|W_full| ~ 13, exp finite in f32); normalization by 1/sumexp
            # is deferred: folded into att_out at the end, and applied to
            # att_sc off the critical path.
            sc_row = rowp.tile([1, l], F32, tag="sc_row", name=f"sc_row_{b}")
            separt = rowp.tile([1, NLT], F32, tag="separt", name=f"separt_{b}")
            for t4 in range(NLT):
                psc = ps_sc.tile([1, NSC], F32, tag="sc", name=f"psc_{b}_{t4}")
                for j in range(AC):
                    nc.tensor.matmul(
                        psc,
                        lhsT=wf_sb[:, j:j + 1],
                        rhs=joined[:, j, t4 * NSC:(t4 + 1) * NSC],
                        start=(j == 0),
                        stop=(j == AC - 1),
                    )
                nc.scalar.activation(
                    out=sc_row[:, t4 * NSC:(t4 + 1) * NSC], in_=psc,
                    func=AF.Exp, bias=zero1, scale=1.0,
                    accum_out=separt[:, t4:t4 + 1],
                )

            # bf16 copy of UNNORMALIZED exp row, bounced to DRAM for broadcast
            sc_bf = rowp.tile([1, l], BF16, tag="sc_bf", name=f"sc_bf_{b}")
            nc.vector.tensor_copy(out=sc_bf, in_=sc_row)
            row_dram = dramp.tile([1, l], BF16, tag="row_dram", name=f"rowd_{b}")
            nc.scalar.dma_start(out=row_dram, in_=sc_bf)

            # sumexp -> DRAM (gathered at the end for att_out normalization);
            # normalized att_sc written off the critical path.
            sumexp = rowp.tile([1, 1], F32, tag="sumexp", name=f"sumexp_{b}")
            nc.vector.tensor_reduce(out=sumexp, in_=separt,
                                    axis=mybir.AxisListType.X, op=ALU.add)
            nc.scalar.dma_start(out=se_dram[b:b + 1, :], in_=sumexp)
            rsum = rowp.tile([1, 1], F32, tag="rsum", name=f"rsum_{b}")
            nc.vector.reciprocal(out=rsum, in_=sumexp)
            nc.vector.tensor_scalar_mul(sc_row, sc_row, rsum)
            nc.scalar.dma_start(out=att_sc[b:b + 1, :], in_=sc_row)
            bc = bcp.tile([P, l], BF16, tag="bc", name=f"bc_{b}")
            bc_src = bass.AP(tensor=row_dram.tensor, offset=row_dram.offset,
                             ap=[[0, P], [1, l]])
            if BC_ENGINE == "scalar":
                nc.scalar.dma_start(out=bc, in_=bc_src)
            else:
                nc.gpsimd.dma_start(out=bc, in_=bc_src)

            # ctx_unnorm[e] = sum_l exp_l * sem[b, l, e]: DVE mults (bf16 2x,
            # L-halves for finer WAR granularity on semT), then free-dim
            # reductions split between ACT (activation accum_out, a free
            # engine) and DVE (tensor_reduce, 1x) to balance load.
            LH = LT // 2
            ctxcol = ttrp.tile([P, EC], F32, tag="ctxcol", name=f"ctxcol_{b}")
            scratch = ttrp.tile([P, 2, LT, P], BF16, tag="scratch",
                                name=f"scr_{b}")
            bc2 = bass.AP(tensor=bc.tensor, offset=bc.offset,
                          ap=[list(bc.ap[0]), [0, 2], [P, LH], [1, P]])
            bc2h = bass.AP(tensor=bc.tensor, offset=bc.offset + LH * P,
                           ap=[list(bc.ap[0]), [0, 2], [P, LH], [1, P]])
            for cp in range(EC // 2):
                for hf, bch in ((0, bc2), (1, bc2h)):
                    in0 = semT[:, hf * LH:(hf + 1) * LH,
                               2 * cp:2 * cp + 2, :].rearrange(
                        "p t c q -> p c t q")
                    nc.vector.tensor_mul(
                        scratch[:, :, hf * LH:(hf + 1) * LH, :], in0, bch)
                for ci in range(2):
                    c = 2 * cp + ci
                    flat = scratch[:, ci, :, :].rearrange("p t q -> p (t q)")
                    if ci == 0:
                        nc.scalar.activation(
                            out=flat, in_=flat, func=AF.Identity,
                            bias=zero1b, scale=1.0,
                            accum_out=ctxcol[:, c:c + 1],
                        )
                    else:
                        nc.vector.tensor_reduce(
                            out=ctxcol[:, c:c + 1], in_=flat,
                            axis=mybir.AxisListType.X, op=ALU.add,
                        )
            nc.vector.tensor_copy(out=ctx_all[:, :, b], in_=ctxcol)

        # ---- att_out = ctx.T @ W_enc + b_enc, batched over blc ----
        ps_att = ps_enc.tile([blc, a], F32, tag="enc")
        for c in range(EC):
            nc.tensor.matmul(
                ps_att,
                lhsT=ctx_all[:, c, :],
                rhs=w_enc_sb[:, c, :],
                start=(c == 0),
                stop=(c == EC - 1),
            )
        se_all = consts.tile([blc, 1], F32)
        nc.sync.dma_start(out=se_all, in_=se_dram)
        rse_all = consts.tile([blc, 1], F32)
        nc.vector.reciprocal(out=rse_all, in_=se_all)
        att_final = consts.tile([blc, a], F32)
        nc.vector.tensor_scalar(att_final, ps_att, rse_all, None, op0=ALU.mult)
        nc.vector.tensor_add(att_final, att_final, b_enc_row)
        nc.sync.dma_start(out=att_out, in_=att_final)

    nc.compile()
    return nc


_CACHE = {}


def _get_nc(blc, l, e, a, h, num_devices):
    key = (blc, l, e, a, h, num_devices)
    if key not in _CACHE:
        _CACHE[key] = build_bass(blc, l, e, a, h, num_devices=num_devices)
    return _CACHE[key]


def make_in_maps(sem_enc_output, dec_hidden_state, W_enc, b_enc, W_dec, b_dec,
                 W_full, n_cores):
    B = sem_enc_output.shape[0]
    blc = B // n_cores
    bf = ml_dtypes.bfloat16
    w_enc_bf = np.ascontiguousarray(W_enc, np.float32).astype(bf)
    w_dec_bf = np.ascontiguousarray(W_dec, np.float32).astype(bf)
    wf_bf = np.ascontiguousarray(W_full[:, 0], np.float32).astype(bf)
    b_enc = np.ascontiguousarray(b_enc, np.float32)
    b_dec = np.ascontiguousarray(b_dec, np.float32)
    sem = np.ascontiguousarray(sem_enc_output, np.float32)
    dec = np.ascontiguousarray(dec_hidden_state, np.float32)
    in_maps = []
    for i in range(n_cores):
        sl = slice(i * blc, (i + 1) * blc)
        in_maps.append({
            "sem": sem[sl],
            "dect_bf": np.ascontiguousarray(dec[sl].T).astype(bf),
            "w_enc_bf": w_enc_bf,
            "w_dec_bf": w_dec_bf,
            "b_enc": b_enc,
            "b_dec": b_dec,
            "wf_bf": wf_bf,
        })
    return in_maps


def kernel(sem_enc_output, dec_hidden_state, W_enc, b_enc, W_dec, b_dec,
           W_full, b_full, _trace=False):
    B, L, E = sem_enc_output.shape
    H = dec_hidden_state.shape[1]
    A = W_enc.shape[1]
    n_cores = 8
    blc = B // n_cores

    nc = _get_nc(blc, L, E, A, H, n_cores)
    in_maps = make_in_maps(sem_enc_output, dec_hidden_state, W_enc, b_enc,
                           W_dec, b_dec, W_full, n_cores)
    res = bass_utils.run_bass_kernel_spmd(
        nc, in_maps, core_ids=list(range(n_cores)), trace=_trace,
    )
    att_out = np.concatenate([r["att_out"] for r in res.results], axis=0)
    att_sc = np.concatenate([r["att_sc"] for r in res.results], axis=0)
    kernel.last_results = res
    return att_out.astype(np.float32), att_sc.astype(np.float32)


# revision 19
# speedup vs baseline: 1.0108x; 1.0108x over previous
"""Trainium2 Bass kernel for the AttentionSemantic module.

Computation (per batch b):
    enc_proj = sem[b] @ W_enc + b_enc                     # [L, A]
    dec_proj = dec[b] @ W_dec + b_dec                     # [A]
    joined   = tanh(enc_proj + dec_proj)                  # [L, A]
    scores   = joined @ W_full (+ b_full, cancels in softmax)
    att_sc   = softmax(scores)                            # [L]
    att_out  = att_sc @ enc_proj                          # [A]
             = (att_sc @ sem[b]) @ W_enc + b_enc          # (sum att_sc == 1)

Sharding: data-parallel over batch B=64 across 8 cores (8 batches/core).

Device strategy (per core):
  - sem[b] cast-loaded f32->bf16 (SWDGE/gpsimd) in natural [L,E] tiles, then
    xbar-DMA-transposed (HWDGE/sync only) into semT [E(part), L(free)] bf16.
    Loads for batch b+1 are emitted before batch b's compute (SW pipeline)
    so the PE never waits on softmax-gated DMAs.
  - enc matmul: PSUM[a_chunk, l_tile] += W_enc[e_chunk, a_chunk].T @ semT,
    loop order (a_chunk, e_chunk, l_tile) to reuse the stationary operand.
  - tanh+bias fused on ACT straight out of PSUM, joined stored bf16 [A, L].
  - scores: M=1 matmuls wf.T @ joined accumulated over a_chunks.
  - softmax on one partition row [1, L] (reduce_max / Exp+accum / mul).
  - ctx = att_sc @ sem[b]: scores row bounced via DRAM (scalar-HWDGE queue),
    broadcast-loaded to 128 partitions, then DVE mult + free-dim reduce
    against semT (bf16 2x mode, chunks paired).
  - att_out = ctx.T @ W_enc + b_enc as one batched M=8 matmul at the end.
"""

import sys

if "/opt/trn_rl_repo" not in sys.path:
    sys.path.insert(0, "/opt/trn_rl_repo")

from contextlib import ExitStack

import ml_dtypes
import numpy as np

import concourse.bass as bass
import concourse.bacc as bacc
import concourse.tile as tile
from concourse import mybir
from concourse import bass_utils

P = 128
F32 = mybir.dt.float32
BF16 = mybir.dt.bfloat16
AF = mybir.ActivationFunctionType
ALU = mybir.AluOpType

# risk fallbacks (validated on hardware)
BC_ENGINE = "scalar"   # "scalar" (HWDGE) or "gpsimd" (SWDGE) broadcast load
RED_BF16 = True        # paired bf16-out reduce (2x) vs f32-out per chunk (1x)
VARIANT = "full"


def build_bass(blc, l, e, a, h, num_devices=8):
    """Build the per-core Bass program. blc = batches per core."""
    LT = l // P        # L tiles of 128 rows
    EC = e // P        # E chunks of 128
    AC = a // P        # A chunks of 128
    HC = h // P        # H chunks of 128
    NSC = 512 if l % 512 == 0 else l   # matmul moving-dim tile along L
    NLT = l // NSC

    nc = bacc.Bacc(
        "TRN2",
        target_bir_lowering=False,
        debug=False,
        enable_asserts=False,
        num_devices=num_devices,
    )

    sem = nc.dram_tensor("sem", [blc, l, e], F32, kind="ExternalInput").ap()
    dect_bf = nc.dram_tensor("dect_bf", [h, blc], BF16, kind="ExternalInput").ap()
    w_enc_bf = nc.dram_tensor("w_enc_bf", [e, a], BF16, kind="ExternalInput").ap()
    w_dec_bf = nc.dram_tensor("w_dec_bf", [h, a], BF16, kind="ExternalInput").ap()
    b_enc = nc.dram_tensor("b_enc", [a], F32, kind="ExternalInput").ap()
    b_dec = nc.dram_tensor("b_dec", [a], F32, kind="ExternalInput").ap()
    wf_bf = nc.dram_tensor("wf_bf", [a], BF16, kind="ExternalInput").ap()

    att_out = nc.dram_tensor("att_out", [blc, a], F32, kind="ExternalOutput").ap()
    att_sc = nc.dram_tensor("att_sc", [blc, l], F32, kind="ExternalOutput").ap()

    with tile.TileContext(nc) as tc, ExitStack() as ctx:
        consts = ctx.enter_context(tc.tile_pool(name="consts", bufs=1))
        natp = ctx.enter_context(tc.tile_pool(name="natp", bufs=3))
        semtp = ctx.enter_context(tc.tile_pool(name="semtp", bufs=3))
        joinp = ctx.enter_context(tc.tile_pool(name="joinp", bufs=1))
        rowp = ctx.enter_context(tc.tile_pool(name="rowp", bufs=1))
        bcp = ctx.enter_context(tc.tile_pool(name="bcp", bufs=2))
        ttrp = ctx.enter_context(tc.tile_pool(name="ttrp", bufs=1))
        dramp = ctx.enter_context(tc.tile_pool(name="dramp", bufs=2, space="DRAM"))
        ps_enc = ctx.enter_context(tc.tile_pool(name="ps_enc", bufs=6, space="PSUM"))
        ps_sc = ctx.enter_context(tc.tile_pool(name="ps_sc", bufs=2, space="PSUM"))

        # ---- constants / params ----
        w_enc_sb = consts.tile([P, EC, a], BF16)
        nc.sync.dma_start(out=w_enc_sb, in_=w_enc_bf.rearrange("(c p) a -> p c a", p=P))
        w_dec_sb = consts.tile([P, HC, a], BF16)
        nc.sync.dma_start(out=w_dec_sb, in_=w_dec_bf.rearrange("(c p) a -> p c a", p=P))
        wf_sb = consts.tile([P, AC], BF16)
        nc.sync.dma_start(out=wf_sb, in_=wf_bf.rearrange("(c p) -> p c", p=P))
        b_enc_sb = consts.tile([P, AC], F32)
        nc.sync.dma_start(out=b_enc_sb, in_=b_enc.rearrange("(c p) -> p c", p=P))
        b_dec_sb = consts.tile([P, AC], F32)
        nc.sync.dma_start(out=b_dec_sb, in_=b_dec.rearrange("(c p) -> p c", p=P))
        bsum = consts.tile([P, AC], F32)
        nc.vector.tensor_add(bsum, b_enc_sb, b_dec_sb)
        dect_sb = consts.tile([P, HC, blc], BF16)
        nc.sync.dma_start(out=dect_sb, in_=dect_bf.rearrange("(c p) b -> p c b", p=P))
        b_enc_row = consts.tile([blc, a], F32)
        nc.gpsimd.dma_start(
            out=b_enc_row,
            in_=bass.AP(tensor=b_enc.tensor, offset=b_enc.offset,
                        ap=[[0, blc], [1, a]]),
        )

        # ---- dec projection: db[a_chunk, b] = (dec @ W_dec + b_dec + b_enc).T ----
        db = consts.tile([P, AC, blc], F32)
        for j in range(AC):
            ps_dec = ps_enc.tile([P, blc], F32, tag="enc", name=f"ps_dec_{j}")
            for c in range(HC):
                nc.tensor.matmul(
                    ps_dec,
                    lhsT=w_dec_sb[:, c, j * P:(j + 1) * P],
                    rhs=dect_sb[:, c, :],
                    start=(c == 0),
                    stop=(c == HC - 1),
                )
            nc.scalar.activation(
                out=db[:, j, :], in_=ps_dec, func=AF.Identity,
                bias=bsum[:, j:j + 1], scale=1.0,
            )

        # ctx (= att_sc @ sem) for all batches, bf16, [e0, e_chunk, b]
        ctx_all = consts.tile([P, EC, blc], BF16)
        zero1 = consts.tile([1, 1], F32)
        nc.vector.memset(zero1, 0.0)
        zero1b = consts.tile([P, 1], F32)
        nc.vector.memset(zero1b, 0.0)
        se_dram = dramp.tile([blc, 1], F32, tag="se_dram", bufs=1)

        # ---- software-pipelined batch loads ----
        # semT layout [e0, t(L-tile), e_chunk, l0] — t-major so each xbar
        # transpose writes a per-partition CONTIGUOUS 2KB run (the l-major
        # layout fragmented the S2M side into 256B descriptors).
        semT_tiles = [None] * blc

        def load_batch(b):
            semT = semtp.tile([P, LT, EC, P], BF16, tag="semT", name=f"semT_{b}")
            semT_tiles[b] = semT
            KG = min(4, LT)
            for tp in range(LT // KG):
                nat = natp.tile([P, KG, e], BF16, tag="nat", name=f"nat_{b}_{tp}")
                nc.gpsimd.dma_start(
                    out=nat,
                    in_=sem[b, tp * KG * P:(tp + 1) * KG * P, :]
                    .rearrange("(k p) e -> p k e", p=P),
                )
                for k in range(KG):
                    t = tp * KG + k
                    nc.sync.dma_start(
                        out=semT[:, t, :, :],
                        in_=nat[:, k, :],
                        transpose=True,
                    )

        load_batch(0)

        # ---- main per-batch loop ----
        for b in range(blc):
            if b + 1 < blc:
                load_batch(b + 1)
            semT = semT_tiles[b]
            semT_tiles[b] = None

            # enc matmul + tanh -> joined bf16 [a0, a_chunk, l]
            joined = joinp.tile([P, AC, l], BF16, tag="joined", name=f"joined_{b}")
            for j in range(AC):
                pss = [
                    ps_enc.tile([P, NSC], F32, tag="enc", name=f"pse_{b}_{j}_{t4}")
                    for t4 in range(NLT)
                ]
                for c in range(EC):
                    for t4 in range(NLT):
                        nc.tensor.matmul(
                            pss[t4],
                            lhsT=w_enc_sb[:, c, j * P:(j + 1) * P],
                            rhs=semT[:, t4 * (NSC // P):(t4 + 1) * (NSC // P), c, :],
                            start=(c == 0),
                            stop=(c == EC - 1),
                        )
                for t4 in range(NLT):
                    nc.scalar.activation(
                        out=joined[:, j, t4 * NSC:(t4 + 1) * NSC],
                        in_=pss[t4], func=AF.Tanh,
                        bias=db[:, j, b:b + 1], scale=1.0,
                    )

            # scores -> exp(scores) straight out of PSUM, per 512-segment,
            # with per-segment partial sums. No max subtraction (|scores| <=
            # sum|W_full| ~ 13, exp finite in f32); normalization by 1/sumexp
            # is deferred: folded into att_out at the end, and applied to
            # att_sc off the critical path.
            sc_row = rowp.tile([1, l], F32, tag="sc_row", name=f"sc_row_{b}")
            separt = rowp.tile([1, NLT], F32, tag="separt", name=f"separt_{b}")
            for t4 in range(NLT):
                psc = ps_sc.tile([1, NSC], F32, tag="sc", name=f"psc_{b}_{t4}")
                for j in range(AC):
                    nc.tensor.matmul(
                        psc,
                        lhsT=wf_sb[:, j:j + 1],
                        rhs=joined[:, j, t4 * NSC:(t4 + 1) * NSC],
                        start=(j == 0),
                        stop=(j == AC - 1),
                    )
                nc.scalar.activation(
                    out=sc_row[:, t4 * NSC:(t4 + 1) * NSC], in_=psc,
                    func=AF.Exp, bias=zero1, scale=1.0,
                    accum_out=separt[:, t4:t4 + 1],
                )

            # bf16 copy of UNNORMALIZED exp row, bounced to DRAM for broadcast
            sc_bf = rowp.tile([1, l], BF16, tag="sc_bf", name=f"sc_bf_{b}")
            nc.vector.tensor_copy(out=sc_bf, in_=sc_row)
            row_dram = dramp.tile([1, l], BF16, tag="row_dram", name=f"rowd_{b}")
            nc.scalar.dma_start(out=row_dram, in_=sc_bf)

            # sumexp -> DRAM (gathered at the end for att_out normalization);
            # normalized att_sc written off the critical path.
            sumexp = rowp.tile([1, 1], F32, tag="sumexp", name=f"sumexp_{b}")
            nc.vector.tensor_reduce(out=sumexp, in_=separt,
                                    axis=mybir.AxisListType.X, op=ALU.add)
            nc.scalar.dma_start(out=se_dram[b:b + 1, :], in_=sumexp)
            rsum = rowp.tile([1, 1], F32, tag="rsum", name=f"rsum_{b}")
            nc.vector.reciprocal(out=rsum, in_=sumexp)
            nc.vector.tensor_scalar_mul(sc_row, sc_row, rsum)
            nc.scalar.dma_start(out=att_sc[b:b + 1, :], in_=sc_row)
            bc = bcp.tile([P, l], BF16, tag="bc", name=f"bc_{b}")
            bc_src = bass.AP(tensor=row_dram.tensor, offset=row_dram.offset,
                             ap=[[0, P], [1, l]])
            if BC_ENGINE == "scalar":
                nc.scalar.dma_start(out=bc, in_=bc_src)
            else:
                nc.gpsimd.dma_start(out=bc, in_=bc_src)

            # ctx_unnorm[e] = sum_l exp_l * sem[b, l, e]: DVE mults (bf16 2x,
            # L-halves for finer WAR granularity on semT), then free-dim
            # reductions split between ACT (activation accum_out, a free
            # engine) and DVE (tensor_reduce, 1x) to balance load.
            LH = LT // 2
            ctxcol = ttrp.tile([P, EC], F32, tag="ctxcol", name=f"ctxcol_{b}")
            sdump = ttrp.tile([P, l], BF16, tag="sdump", name=f"sdump_{b}")
            scratch = ttrp.tile([P, 2, LT, P], BF16, tag="scratch",
                                name=f"scr_{b}")
            bc2 = bass.AP(tensor=bc.tensor, offset=bc.offset,
                          ap=[list(bc.ap[0]), [0, 2], [P, LH], [1, P]])
            bc2h = bass.AP(tensor=bc.tensor, offset=bc.offset + LH * P,
                           ap=[list(bc.ap[0]), [0, 2], [P, LH], [1, P]])
            for cp in range(EC // 2):
                for hf, bch in ((0, bc2), (1, bc2h)):
                    in0 = semT[:, hf * LH:(hf + 1) * LH,
                               2 * cp:2 * cp + 2, :].rearrange(
                        "p t c q -> p c t q")
                    nc.vector.tensor_mul(
                        scratch[:, :, hf * LH:(hf + 1) * LH, :], in0, bch)
                for ci in range(2):
                    c = 2 * cp + ci
                    flat = scratch[:, ci, :, :].rearrange("p t q -> p (t q)")
                    if ci == 0:
                        nc.scalar.activation(
                            out=sdump, in_=flat, func=AF.Identity,
                            bias=zero1b, scale=1.0,
                            accum_out=ctxcol[:, c:c + 1],
                        )
                    else:
                        nc.vector.tensor_reduce(
                            out=ctxcol[:, c:c + 1], in_=flat,
                            axis=mybir.AxisListType.X, op=ALU.add,
                        )
            nc.vector.tensor_copy(out=ctx_all[:, :, b], in_=ctxcol)

        # ---- att_out = ctx.T @ W_enc + b_enc, batched over blc ----
        ps_att = ps_enc.tile([blc, a], F32, tag="enc")
        for c in range(EC):
            nc.tensor.matmul(
                ps_att,
                lhsT=ctx_all[:, c, :],
                rhs=w_enc_sb[:, c, :],
                start=(c == 0),
                stop=(c == EC - 1),
            )
        se_all = consts.tile([blc, 1], F32)
        nc.sync.dma_start(out=se_all, in_=se_dram)
        rse_all = consts.tile([blc, 1], F32)
        nc.vector.reciprocal(out=rse_all, in_=se_all)
        att_final = consts.tile([blc, a], F32)
        nc.vector.tensor_scalar(att_final, ps_att, rse_all, None, op0=ALU.mult)
        nc.vector.tensor_add(att_final, att_final, b_enc_row)
        nc.sync.dma_start(out=att_out, in_=att_final)

    nc.compile()
    return nc


_CACHE = {}


def _get_nc(blc, l, e, a, h, num_devices):
    key = (blc, l, e, a, h, num_devices)
    if key not in _CACHE:
        _CACHE[key] = build_bass(blc, l, e, a, h, num_devices=num_devices)
    return _CACHE[key]


def make_in_maps(sem_enc_output, dec_hidden_state, W_enc, b_enc, W_dec, b_dec,
                 W_full, n_cores):
    B = sem_enc_output.shape[0]
    blc = B // n_cores
    bf = ml_dtypes.bfloat16
    w_enc_bf = np.ascontiguousarray(W_enc, np.float32).astype(bf)
    w_dec_bf = np.ascontiguousarray(W_dec, np.float32).astype(bf)
    wf_bf = np.ascontiguousarray(W_full[:, 0], np.float32).astype(bf)
    b_enc = np.ascontiguousarray(b_enc, np.float32)
    b_dec = np.ascontiguousarray(b_dec, np.float32)
    sem = np.ascontiguousarray(sem_enc_output, np.float32)
    dec = np.ascontiguousarray(dec_hidden_state, np.float32)
    in_maps = []
    for i in range(n_cores):
        sl = slice(i * blc, (i + 1) * blc)
        in_maps.append({
            "sem": sem[sl],
            "dect_bf": np.ascontiguousarray(dec[sl].T).astype(bf),
            "w_enc_bf": w_enc_bf,
            "w_dec_bf": w_dec_bf,
            "b_enc": b_enc,
            "b_dec": b_dec,
            "wf_bf": wf_bf,
        })
    return in_maps


def kernel(sem_enc_output, dec_hidden_state, W_enc, b_enc, W_dec, b_dec,
           W_full, b_full, _trace=False):
    B, L, E = sem_enc_output.shape
    H = dec_hidden_state.shape[1]
    A = W_enc.shape[1]
    n_cores = 8
    blc = B // n_cores

    nc = _get_nc(blc, L, E, A, H, n_cores)
    in_maps = make_in_maps(sem_enc_output, dec_hidden_state, W_enc, b_enc,
                           W_dec, b_dec, W_full, n_cores)
    res = bass_utils.run_bass_kernel_spmd(
        nc, in_maps, core_ids=list(range(n_cores)), trace=_trace,
    )
    att_out = np.concatenate([r["att_out"] for r in res.results], axis=0)
    att_sc = np.concatenate([r["att_sc"] for r in res.results], axis=0)
    kernel.last_results = res
    return att_out.astype(np.float32), att_sc.astype(np.float32)


# revision 22
# speedup vs baseline: 1.0451x; 1.0339x over previous
"""Trainium2 Bass kernel for the AttentionSemantic module.

Computation (per batch b):
    enc_proj = sem[b] @ W_enc + b_enc                     # [L, A]
    dec_proj = dec[b] @ W_dec + b_dec                     # [A]
    joined   = tanh(enc_proj + dec_proj)                  # [L, A]
    scores   = joined @ W_full (+ b_full, cancels in softmax)
    att_sc   = softmax(scores)                            # [L]
    att_out  = att_sc @ enc_proj                          # [A]
             = (att_sc @ sem[b]) @ W_enc + b_enc          # (sum att_sc == 1)

Sharding: data-parallel over batch B=64 across 8 cores (8 batches/core).

Device strategy (per core):
  - sem[b] cast-loaded f32->bf16 (SWDGE/gpsimd) in natural [L,E] tiles, then
    xbar-DMA-transposed (HWDGE/sync, its own queue) into semT
    [E(part), t, e_chunk, l0] bf16. Loads run two batches ahead of compute
    (depth-2 software pipeline) so the DMA cascade never idles at batch
    boundaries and the PE is never starved by softmax-gated DMAs.
  - enc matmul: PSUM[a_chunk, l_tile] += W_enc[e_chunk, a_chunk].T @ semT,
    loop order (a_chunk, e_chunk, l_tile) to reuse the stationary operand.
  - tanh+bias fused on ACT straight out of PSUM, joined stored bf16 [A, L].
  - scores: M=1 matmuls wf.T @ joined; exp() fused on the PSUM->SBUF copy
    with per-segment accum_out partial sums (no max subtraction: scores are
    tanh-bounded, exp stays finite in f32). Softmax normalization deferred:
    1/sumexp folded into att_out at the end; att_sc normalized off-path.
  - ctx_unnorm = exp_row @ sem[b]: exp row bounced via DRAM (scalar-HWDGE
    queue) and broadcast-loaded to 128 partitions; DVE bf16 2x mults against
    semT, reductions split between ACT (activation accum_out) and DVE.
  - att_out = (ctx.T @ W_enc) / sumexp + b_enc, one batched M=8 matmul.
"""

import sys

if "/opt/trn_rl_repo" not in sys.path:
    sys.path.insert(0, "/opt/trn_rl_repo")

from contextlib import ExitStack

import ml_dtypes
import numpy as np

import concourse.bass as bass
import concourse.bacc as bacc
import concourse.tile as tile
from concourse import mybir
from concourse import bass_utils

P = 128
F32 = mybir.dt.float32
BF16 = mybir.dt.bfloat16
AF = mybir.ActivationFunctionType
ALU = mybir.AluOpType

# risk fallbacks (validated on hardware)
BC_ENGINE = "scalar"   # "scalar" (HWDGE) or "gpsimd" (SWDGE) broadcast load
RED_BF16 = True        # paired bf16-out reduce (2x) vs f32-out per chunk (1x)
VARIANT = "full"


def build_bass(blc, l, e, a, h, num_devices=8):
    """Build the per-core Bass program. blc = batches per core."""
    LT = l // P        # L tiles of 128 rows
    EC = e // P        # E chunks of 128
    AC = a // P        # A chunks of 128
    HC = h // P        # H chunks of 128
    NSC = 512 if l % 512 == 0 else l   # matmul moving-dim tile along L
    NLT = l // NSC

    nc = bacc.Bacc(
        "TRN2",
        target_bir_lowering=False,
        debug=False,
        enable_asserts=False,
        num_devices=num_devices,
    )

    sem = nc.dram_tensor("sem", [blc, l, e], F32, kind="ExternalInput").ap()
    dect_bf = nc.dram_tensor("dect_bf", [h, blc], BF16, kind="ExternalInput").ap()
    w_enc_bf = nc.dram_tensor("w_enc_bf", [e, a], BF16, kind="ExternalInput").ap()
    w_dec_bf = nc.dram_tensor("w_dec_bf", [h, a], BF16, kind="ExternalInput").ap()
    b_enc = nc.dram_tensor("b_enc", [a], F32, kind="ExternalInput").ap()
    b_dec = nc.dram_tensor("b_dec", [a], F32, kind="ExternalInput").ap()
    wf_bf = nc.dram_tensor("wf_bf", [a], BF16, kind="ExternalInput").ap()

    att_out = nc.dram_tensor("att_out", [blc, a], F32, kind="ExternalOutput").ap()
    att_sc = nc.dram_tensor("att_sc", [blc, l], F32, kind="ExternalOutput").ap()

    with tile.TileContext(nc) as tc, ExitStack() as ctx:
        consts = ctx.enter_context(tc.tile_pool(name="consts", bufs=1))
        natp = ctx.enter_context(tc.tile_pool(name="natp", bufs=2))
        semtp = ctx.enter_context(tc.tile_pool(name="semtp", bufs=3))
        joinp = ctx.enter_context(tc.tile_pool(name="joinp", bufs=1))
        rowp = ctx.enter_context(tc.tile_pool(name="rowp", bufs=1))
        bcp = ctx.enter_context(tc.tile_pool(name="bcp", bufs=2))
        ttrp = ctx.enter_context(tc.tile_pool(name="ttrp", bufs=1))
        dramp = ctx.enter_context(tc.tile_pool(name="dramp", bufs=2, space="DRAM"))
        ps_enc = ctx.enter_context(tc.tile_pool(name="ps_enc", bufs=6, space="PSUM"))
        ps_sc = ctx.enter_context(tc.tile_pool(name="ps_sc", bufs=2, space="PSUM"))

        # ---- constants / params ----
        w_enc_sb = consts.tile([P, EC, a], BF16)
        nc.sync.dma_start(out=w_enc_sb, in_=w_enc_bf.rearrange("(c p) a -> p c a", p=P))
        w_dec_sb = consts.tile([P, HC, a], BF16)
        nc.sync.dma_start(out=w_dec_sb, in_=w_dec_bf.rearrange("(c p) a -> p c a", p=P))
        wf_sb = consts.tile([P, AC], BF16)
        nc.sync.dma_start(out=wf_sb, in_=wf_bf.rearrange("(c p) -> p c", p=P))
        b_enc_sb = consts.tile([P, AC], F32)
        nc.sync.dma_start(out=b_enc_sb, in_=b_enc.rearrange("(c p) -> p c", p=P))
        b_dec_sb = consts.tile([P, AC], F32)
        nc.sync.dma_start(out=b_dec_sb, in_=b_dec.rearrange("(c p) -> p c", p=P))
        bsum = consts.tile([P, AC], F32)
        nc.vector.tensor_add(bsum, b_enc_sb, b_dec_sb)
        dect_sb = consts.tile([P, HC, blc], BF16)
        nc.sync.dma_start(out=dect_sb, in_=dect_bf.rearrange("(c p) b -> p c b", p=P))
        b_enc_row = consts.tile([blc, a], F32)
        nc.gpsimd.dma_start(
            out=b_enc_row,
            in_=bass.AP(tensor=b_enc.tensor, offset=b_enc.offset,
                        ap=[[0, blc], [1, a]]),
        )

        # ---- dec projection: db[a_chunk, b] = (dec @ W_dec + b_dec + b_enc).T ----
        db = consts.tile([P, AC, blc], F32)
        for j in range(AC):
            ps_dec = ps_enc.tile([P, blc], F32, tag="enc", name=f"ps_dec_{j}")
            for c in range(HC):
                nc.tensor.matmul(
                    ps_dec,
                    lhsT=w_dec_sb[:, c, j * P:(j + 1) * P],
                    rhs=dect_sb[:, c, :],
                    start=(c == 0),
                    stop=(c == HC - 1),
                )
            nc.scalar.activation(
                out=db[:, j, :], in_=ps_dec, func=AF.Identity,
                bias=bsum[:, j:j + 1], scale=1.0,
            )

        # ctx (= att_sc @ sem) for all batches, bf16, [e0, e_chunk, b]
        ctx_all = consts.tile([P, EC, blc], BF16)
        zero1 = consts.tile([1, 1], F32)
        nc.vector.memset(zero1, 0.0)
        zero1b = consts.tile([P, 1], F32)
        nc.vector.memset(zero1b, 0.0)
        se_dram = dramp.tile([blc, 1], F32, tag="se_dram", bufs=1)

        # ---- software-pipelined batch loads ----
        # semT layout [e0, t(L-tile), e_chunk, l0] — t-major so each xbar
        # transpose writes a per-partition CONTIGUOUS 2KB run (the l-major
        # layout fragmented the S2M side into 256B descriptors).
        semT_tiles = [None] * blc

        def load_batch(b):
            semT = semtp.tile([P, LT, EC, P], BF16, tag="semT", name=f"semT_{b}")
            semT_tiles[b] = semT
            KG = min(4, LT)
            for tp in range(LT // KG):
                nat = natp.tile([P, KG, e], BF16, tag="nat", name=f"nat_{b}_{tp}")
                nc.gpsimd.dma_start(
                    out=nat,
                    in_=sem[b, tp * KG * P:(tp + 1) * KG * P, :]
                    .rearrange("(k p) e -> p k e", p=P),
                )
                for k in range(KG):
                    t = tp * KG + k
                    nc.sync.dma_start(
                        out=semT[:, t, :, :],
                        in_=nat[:, k, :],
                        transpose=True,
                    )

        load_batch(0)
        if blc > 1:
            load_batch(1)

        # ---- main per-batch loop ----
        for b in range(blc):
            if b + 2 < blc:
                load_batch(b + 2)
            semT = semT_tiles[b]
            semT_tiles[b] = None

            # enc matmul + tanh -> joined bf16 [a0, a_chunk, l]
            joined = joinp.tile([P, AC, l], BF16, tag="joined", name=f"joined_{b}")
            for j in range(AC):
                pss = [
                    ps_enc.tile([P, NSC], F32, tag="enc", name=f"pse_{b}_{j}_{t4}")
                    for t4 in range(NLT)
                ]
                for c in range(EC):
                    for t4 in range(NLT):
                        nc.tensor.matmul(
                            pss[t4],
                            lhsT=w_enc_sb[:, c, j * P:(j + 1) * P],
                            rhs=semT[:, t4 * (NSC // P):(t4 + 1) * (NSC // P), c, :],
                            start=(c == 0),
                            stop=(c == EC - 1),
                        )
                for t4 in range(NLT):
                    nc.scalar.activation(
                        out=joined[:, j, t4 * NSC:(t4 + 1) * NSC],
                        in_=pss[t4], func=AF.Tanh,
                        bias=db[:, j, b:b + 1], scale=1.0,
                    )

            # scores -> exp(scores) straight out of PSUM, per 512-segment,
            # with per-segment partial sums. No max subtraction (|scores| <=
            # sum|W_full| ~ 13, exp finite in f32); normalization by 1/sumexp
            # is deferred: folded into att_out at the end, and applied to
            # att_sc off the critical path.
            sc_row = rowp.tile([1, l], F32, tag="sc_row", name=f"sc_row_{b}")
            separt = rowp.tile([1, NLT], F32, tag="separt", name=f"separt_{b}")
            for t4 in range(NLT):
                psc = ps_sc.tile([1, NSC], F32, tag="sc", name=f"psc_{b}_{t4}")
                for j in range(AC):
                    nc.tensor.matmul(
                        psc,
                        lhsT=wf_sb[:, j:j + 1],
                        rhs=joined[:, j, t4 * NSC:(t4 + 1) * NSC],
                        start=(j == 0),
                        stop=(j == AC - 1),
                    )
                nc.scalar.activation(
                    out=sc_row[:, t4 * NSC:(t4 + 1) * NSC], in_=psc,
                    func=AF.Exp, bias=zero1, scale=1.0,
                    accum_out=separt[:, t4:t4 + 1],
                )

            # bf16 copy of UNNORMALIZED exp row, bounced to DRAM for broadcast
            sc_bf = rowp.tile([1, l], BF16, tag="sc_bf", name=f"sc_bf_{b}")
            nc.vector.tensor_copy(out=sc_bf, in_=sc_row)
            row_dram = dramp.tile([1, l], BF16, tag="row_dram", name=f"rowd_{b}")
            nc.scalar.dma_start(out=row_dram, in_=sc_bf)

            # sumexp -> DRAM (gathered at the end for att_out normalization);
            # normalized att_sc written off the critical path.
            sumexp = rowp.tile([1, 1], F32, tag="sumexp", name=f"sumexp_{b}")
            nc.vector.tensor_reduce(out=sumexp, in_=separt,
                                    axis=mybir.AxisListType.X, op=ALU.add)
            nc.scalar.dma_start(out=se_dram[b:b + 1, :], in_=sumexp)
            rsum = rowp.tile([1, 1], F32, tag="rsum", name=f"rsum_{b}")
            nc.vector.reciprocal(out=rsum, in_=sumexp)
            nc.vector.tensor_scalar_mul(sc_row, sc_row, rsum)
            nc.scalar.dma_start(out=att_sc[b:b + 1, :], in_=sc_row)
            bc = bcp.tile([P, l], BF16, tag="bc", name=f"bc_{b}")
            bc_src = bass.AP(tensor=row_dram.tensor, offset=row_dram.offset,
                             ap=[[0, P], [1, l]])
            if BC_ENGINE == "scalar":
                nc.scalar.dma_start(out=bc, in_=bc_src)
            else:
                nc.gpsimd.dma_start(out=bc, in_=bc_src)

            # ctx_unnorm[e] = sum_l exp_l * sem[b, l, e]: DVE mults (bf16 2x,
            # L-halves for finer WAR granularity on semT), then free-dim
            # reductions split between ACT (activation accum_out, a free
            # engine) and DVE (tensor_reduce, 1x) to balance load.
            LH = LT // 2
            ctxcol = ttrp.tile([P, EC], F32, tag="ctxcol", name=f"ctxcol_{b}")
            sdump = ttrp.tile([P, l], BF16, tag="sdump", name=f"sdump_{b}")
            scratch = ttrp.tile([P, 2, LT, P], BF16, tag="scratch",
                                name=f"scr_{b}")
            bc2 = bass.AP(tensor=bc.tensor, offset=bc.offset,
                          ap=[list(bc.ap[0]), [0, 2], [P, LH], [1, P]])
            bc2h = bass.AP(tensor=bc.tensor, offset=bc.offset + LH * P,
                           ap=[list(bc.ap[0]), [0, 2], [P, LH], [1, P]])
            for cp in range(EC // 2):
                for hf, bch in ((0, bc2), (1, bc2h)):
                    in0 = semT[:, hf * LH:(hf + 1) * LH,
                               2 * cp:2 * cp + 2, :].rearrange(
                        "p t c q -> p c t q")
                    nc.vector.tensor_mul(
                        scratch[:, :, hf * LH:(hf + 1) * LH, :], in0, bch)
                for ci in range(2):
                    c = 2 * cp + ci
                    flat = scratch[:, ci, :, :].rearrange("p t q -> p (t q)")
                    if ci == 0:
                        nc.scalar.activation(
                            out=sdump, in_=flat, func=AF.Identity,
                            bias=zero1b, scale=1.0,
                            accum_out=ctxcol[:, c:c + 1],
                        )
                    else:
                        nc.vector.tensor_reduce(
                            out=ctxcol[:, c:c + 1], in_=flat,
                            axis=mybir.AxisListType.X, op=ALU.add,
                        )
            nc.vector.tensor_copy(out=ctx_all[:, :, b], in_=ctxcol)

        # ---- att_out = ctx.T @ W_enc + b_enc, batched over blc ----
        ps_att = ps_enc.tile([blc, a], F32, tag="enc")
        for c in range(EC):
            nc.tensor.matmul(
                ps_att,
                lhsT=ctx_all[:, c, :],
                rhs=w_enc_sb[:, c, :],
                start=(c == 0),
                stop=(c == EC - 1),
            )
        se_all = consts.tile([blc, 1], F32)
        nc.sync.dma_start(out=se_all, in_=se_dram)
        rse_all = consts.tile([blc, 1], F32)
        nc.vector.reciprocal(out=rse_all, in_=se_all)
        att_final = consts.tile([blc, a], F32)
        nc.vector.tensor_scalar(att_final, ps_att, rse_all, None, op0=ALU.mult)
        nc.vector.tensor_add(att_final, att_final, b_enc_row)
        nc.sync.dma_start(out=att_out, in_=att_final)

    nc.compile()
    return nc


_CACHE = {}


def _get_nc(blc, l, e, a, h, num_devices):
    key = (blc, l, e, a, h, num_devices)
    if key not in _CACHE:
        _CACHE[key] = build_bass(blc, l, e, a, h, num_devices=num_devices)
    return _CACHE[key]


def make_in_maps(sem_enc_output, dec_hidden_state, W_enc, b_enc, W_dec, b_dec,
                 W_full, n_cores):
    B = sem_enc_output.shape[0]
    blc = B // n_cores
    bf = ml_dtypes.bfloat16
    w_enc_bf = np.ascontiguousarray(W_enc, np.float32).astype(bf)
    w_dec_bf = np.ascontiguousarray(W_dec, np.float32).astype(bf)
    wf_bf = np.ascontiguousarray(W_full[:, 0], np.float32).astype(bf)
    b_enc = np.ascontiguousarray(b_enc, np.float32)
    b_dec = np.ascontiguousarray(b_dec, np.float32)
    sem = np.ascontiguousarray(sem_enc_output, np.float32)
    dec = np.ascontiguousarray(dec_hidden_state, np.float32)
    in_maps = []
    for i in range(n_cores):
        sl = slice(i * blc, (i + 1) * blc)
        in_maps.append({
            "sem": sem[sl],
            "dect_bf": np.ascontiguousarray(dec[sl].T).astype(bf),
            "w_enc_bf": w_enc_bf,
            "w_dec_bf": w_dec_bf,
            "b_enc": b_enc,
            "b_dec": b_dec,
            "wf_bf": wf_bf,
        })
    return in_maps


def kernel(sem_enc_output, dec_hidden_state, W_enc, b_enc, W_dec, b_dec,
           W_full, b_full, _trace=False):
    B, L, E = sem_enc_output.shape
    H = dec_hidden_state.shape[1]
    A = W_enc.shape[1]
    n_cores = 8
    blc = B // n_cores

    nc = _get_nc(blc, L, E, A, H, n_cores)
    in_maps = make_in_maps(sem_enc_output, dec_hidden_state, W_enc, b_enc,
                           W_dec, b_dec, W_full, n_cores)
    res = bass_utils.run_bass_kernel_spmd(
        nc, in_maps, core_ids=list(range(n_cores)), trace=_trace,
    )
    att_out = np.concatenate([r["att_out"] for r in res.results], axis=0)
    att_sc = np.concatenate([r["att_sc"] for r in res.results], axis=0)
    kernel.last_results = res
    return att_out.astype(np.float32), att_sc.astype(np.float32)


# revision 24
# speedup vs baseline: 1.0822x; 1.0354x over previous
"""Trainium2 Bass kernel for the AttentionSemantic module.

Computation (per batch b):
    enc_proj = sem[b] @ W_enc + b_enc                     # [L, A]
    dec_proj = dec[b] @ W_dec + b_dec                     # [A]
    joined   = tanh(enc_proj + dec_proj)                  # [L, A]
    scores   = joined @ W_full (+ b_full, cancels in softmax)
    att_sc   = softmax(scores)                            # [L]
    att_out  = att_sc @ enc_proj                          # [A]
             = (att_sc @ sem[b]) @ W_enc + b_enc          # (sum att_sc == 1)

Sharding: data-parallel over batch B=64 across 8 cores (8 batches/core).

Device strategy (per core):
  - sem[b] cast-loaded f32->bf16 (SWDGE/gpsimd) in natural [L,E] tiles, then
    xbar-DMA-transposed (HWDGE/sync, its own queue) into semT
    [E(part), t, e_chunk, l0] bf16. Loads for batch b+1 are emitted before
    batch b's compute (software pipeline, semT triple-buffered) so the PE is
    not starved by softmax-gated DMAs.
  - enc matmul: PSUM[a_chunk, l_tile] += W_enc[e_chunk, a_chunk].T @ semT,
    loop order (a_chunk, e_chunk, l_tile) to reuse the stationary operand.
  - tanh+bias fused on ACT straight out of PSUM, joined stored bf16 [A, L].
  - scores: M=1 matmuls wf.T @ joined; exp() fused on the PSUM->SBUF copy
    with per-segment accum_out partial sums (no max subtraction: scores are
    tanh-bounded, exp stays finite in f32). Softmax normalization deferred:
    1/sumexp folded into att_out at the end; att_sc normalized off-path.
  - ctx_unnorm = exp_row @ sem[b]: exp row bounced via DRAM (scalar-HWDGE
    queue) and broadcast-loaded to 128 partitions; DVE bf16 2x mults against
    semT, reductions split between ACT (activation accum_out) and DVE.
  - att_out = (ctx.T @ W_enc) / sumexp + b_enc, one batched M=8 matmul.
"""

import sys

if "/opt/trn_rl_repo" not in sys.path:
    sys.path.insert(0, "/opt/trn_rl_repo")

from contextlib import ExitStack

import ml_dtypes
import numpy as np

import concourse.bass as bass
import concourse.bacc as bacc
import concourse.tile as tile
from concourse import mybir
from concourse import bass_utils

P = 128
F32 = mybir.dt.float32
BF16 = mybir.dt.bfloat16
AF = mybir.ActivationFunctionType
ALU = mybir.AluOpType

# risk fallbacks (validated on hardware)
BC_ENGINE = "scalar"   # "scalar" (HWDGE) or "gpsimd" (SWDGE) broadcast load
RED_BF16 = True        # paired bf16-out reduce (2x) vs f32-out per chunk (1x)
VARIANT = "full"


def build_bass(blc, l, e, a, h, num_devices=8):
    """Build the per-core Bass program. blc = batches per core."""
    LT = l // P        # L tiles of 128 rows
    EC = e // P        # E chunks of 128
    AC = a // P        # A chunks of 128
    HC = h // P        # H chunks of 128
    NSC = 512 if l % 512 == 0 else l   # matmul moving-dim tile along L
    NLT = l // NSC

    nc = bacc.Bacc(
        "TRN2",
        target_bir_lowering=False,
        debug=False,
        enable_asserts=False,
        num_devices=num_devices,
    )

    sem = nc.dram_tensor("sem", [blc, l, e], F32, kind="ExternalInput").ap()
    dect_bf = nc.dram_tensor("dect_bf", [h, blc], BF16, kind="ExternalInput").ap()
    w_enc_bf = nc.dram_tensor("w_enc_bf", [e, a], BF16, kind="ExternalInput").ap()
    w_dec_bf = nc.dram_tensor("w_dec_bf", [h, a], BF16, kind="ExternalInput").ap()
    b_enc = nc.dram_tensor("b_enc", [a], F32, kind="ExternalInput").ap()
    b_dec = nc.dram_tensor("b_dec", [a], F32, kind="ExternalInput").ap()
    wf_bf = nc.dram_tensor("wf_bf", [a], BF16, kind="ExternalInput").ap()

    att_out = nc.dram_tensor("att_out", [blc, a], F32, kind="ExternalOutput").ap()
    att_sc = nc.dram_tensor("att_sc", [blc, l], F32, kind="ExternalOutput").ap()

    with tile.TileContext(nc) as tc, ExitStack() as ctx:
        consts = ctx.enter_context(tc.tile_pool(name="consts", bufs=1))
        natp = ctx.enter_context(tc.tile_pool(name="natp", bufs=2))
        semtp = ctx.enter_context(tc.tile_pool(name="semtp", bufs=3))
        joinp = ctx.enter_context(tc.tile_pool(name="joinp", bufs=2))
        rowp = ctx.enter_context(tc.tile_pool(name="rowp", bufs=1))
        bcp = ctx.enter_context(tc.tile_pool(name="bcp", bufs=2))
        ttrp = ctx.enter_context(tc.tile_pool(name="ttrp", bufs=1))
        dramp = ctx.enter_context(tc.tile_pool(name="dramp", bufs=2, space="DRAM"))
        ps_enc = ctx.enter_context(tc.tile_pool(name="ps_enc", bufs=5, space="PSUM"))
        ps_sc = ctx.enter_context(tc.tile_pool(name="ps_sc", bufs=3, space="PSUM"))

        # ---- constants / params ----
        w_enc_sb = consts.tile([P, EC, a], BF16)
        nc.sync.dma_start(out=w_enc_sb, in_=w_enc_bf.rearrange("(c p) a -> p c a", p=P))
        w_dec_sb = consts.tile([P, HC, a], BF16)
        nc.sync.dma_start(out=w_dec_sb, in_=w_dec_bf.rearrange("(c p) a -> p c a", p=P))
        wf_sb = consts.tile([P, AC], BF16)
        nc.sync.dma_start(out=wf_sb, in_=wf_bf.rearrange("(c p) -> p c", p=P))
        b_enc_sb = consts.tile([P, AC], F32)
        nc.sync.dma_start(out=b_enc_sb, in_=b_enc.rearrange("(c p) -> p c", p=P))
        b_dec_sb = consts.tile([P, AC], F32)
        nc.sync.dma_start(out=b_dec_sb, in_=b_dec.rearrange("(c p) -> p c", p=P))
        bsum = consts.tile([P, AC], F32)
        nc.vector.tensor_add(bsum, b_enc_sb, b_dec_sb)
        dect_sb = consts.tile([P, HC, blc], BF16)
        nc.sync.dma_start(out=dect_sb, in_=dect_bf.rearrange("(c p) b -> p c b", p=P))
        b_enc_row = consts.tile([blc, a], F32)
        nc.gpsimd.dma_start(
            out=b_enc_row,
            in_=bass.AP(tensor=b_enc.tensor, offset=b_enc.offset,
                        ap=[[0, blc], [1, a]]),
        )

        # ---- dec projection: db[a_chunk, b] = (dec @ W_dec + b_dec + b_enc).T ----
        db = consts.tile([P, AC, blc], F32)
        for j in range(AC):
            ps_dec = ps_enc.tile([P, blc], F32, tag="enc", name=f"ps_dec_{j}")
            for c in range(HC):
                nc.tensor.matmul(
                    ps_dec,
                    lhsT=w_dec_sb[:, c, j * P:(j + 1) * P],
                    rhs=dect_sb[:, c, :],
                    start=(c == 0),
                    stop=(c == HC - 1),
                )
            nc.scalar.activation(
                out=db[:, j, :], in_=ps_dec, func=AF.Identity,
                bias=bsum[:, j:j + 1], scale=1.0,
            )

        # ctx (= att_sc @ sem) for all batches, bf16, [e0, e_chunk, b]
        ctx_all = consts.tile([P, EC, blc], BF16)
        zero1 = consts.tile([1, 1], F32)
        nc.vector.memset(zero1, 0.0)
        zero1b = consts.tile([P, 1], F32)
        nc.vector.memset(zero1b, 0.0)
        se_dram = dramp.tile([blc, 1], F32, tag="se_dram", bufs=1)

        # ---- software-pipelined batch loads ----
        # semT layout [e0, t(L-tile), e_chunk, l0] — t-major so each xbar
        # transpose writes a per-partition CONTIGUOUS 2KB run (the l-major
        # layout fragmented the S2M side into 256B descriptors).
        semT_tiles = [None] * blc

        def load_batch(b):
            semT = semtp.tile([P, LT, EC, P], BF16, tag="semT", name=f"semT_{b}")
            semT_tiles[b] = semT
            KG = min(4, LT)
            for tp in range(LT // KG):
                nat = natp.tile([P, KG, e], BF16, tag="nat", name=f"nat_{b}_{tp}")
                nc.gpsimd.dma_start(
                    out=nat,
                    in_=sem[b, tp * KG * P:(tp + 1) * KG * P, :]
                    .rearrange("(k p) e -> p k e", p=P),
                )
                for k in range(KG):
                    t = tp * KG + k
                    nc.sync.dma_start(
                        out=semT[:, t, :, :],
                        in_=nat[:, k, :],
                        transpose=True,
                    )

        load_batch(0)

        # ---- main per-batch loop ----
        for b in range(blc):
            if b + 1 < blc:
                load_batch(b + 1)
            semT = semT_tiles[b]
            semT_tiles[b] = None

            # enc matmul + tanh -> joined bf16 [a0, a_chunk, l]
            joined = joinp.tile([P, AC, l], BF16, tag="joined", name=f"joined_{b}")
            for j in range(AC):
                pss = [
                    ps_enc.tile([P, NSC], F32, tag="enc", name=f"pse_{b}_{j}_{t4}")
                    for t4 in range(NLT)
                ]
                for c in range(EC):
                    for t4 in range(NLT):
                        nc.tensor.matmul(
                            pss[t4],
                            lhsT=w_enc_sb[:, c, j * P:(j + 1) * P],
                            rhs=semT[:, t4 * (NSC // P):(t4 + 1) * (NSC // P), c, :],
                            start=(c == 0),
                            stop=(c == EC - 1),
                        )
                for t4 in range(NLT):
                    nc.scalar.activation(
                        out=joined[:, j, t4 * NSC:(t4 + 1) * NSC],
                        in_=pss[t4], func=AF.Tanh,
                        bias=db[:, j, b:b + 1], scale=1.0,
                    )

            # scores -> exp(scores) straight out of PSUM, per 512-segment,
            # with per-segment partial sums. No max subtraction (|scores| <=
            # sum|W_full| ~ 13, exp finite in f32); normalization by 1/sumexp
            # is deferred: folded into att_out at the end, and applied to
            # att_sc off the critical path.
            sc_row = rowp.tile([1, l], F32, tag="sc_row", name=f"sc_row_{b}")
            separt = rowp.tile([1, NLT], F32, tag="separt", name=f"separt_{b}")
            for t4 in range(NLT):
                psc = ps_sc.tile([1, NSC], F32, tag="sc", name=f"psc_{b}_{t4}")
                for j in range(AC):
                    nc.tensor.matmul(
                        psc,
                        lhsT=wf_sb[:, j:j + 1],
                        rhs=joined[:, j, t4 * NSC:(t4 + 1) * NSC],
                        start=(j == 0),
                        stop=(j == AC - 1),
                    )
                nc.scalar.activation(
                    out=sc_row[:, t4 * NSC:(t4 + 1) * NSC], in_=psc,
                    func=AF.Exp, bias=zero1, scale=1.0,
                    accum_out=separt[:, t4:t4 + 1],
                )

            # bf16 copy of UNNORMALIZED exp row, bounced to DRAM for broadcast
            sc_bf = rowp.tile([1, l], BF16, tag="sc_bf", name=f"sc_bf_{b}")
            nc.vector.tensor_copy(out=sc_bf, in_=sc_row)
            row_dram = dramp.tile([1, l], BF16, tag="row_dram", name=f"rowd_{b}")
            nc.scalar.dma_start(out=row_dram, in_=sc_bf)

            # sumexp -> DRAM (gathered at the end for att_out normalization);
            # normalized att_sc written off the critical path.
            sumexp = rowp.tile([1, 1], F32, tag="sumexp", name=f"sumexp_{b}")
            nc.vector.tensor_reduce(out=sumexp, in_=separt,
                                    axis=mybir.AxisListType.X, op=ALU.add)
            nc.scalar.dma_start(out=se_dram[b:b + 1, :], in_=sumexp)
            rsum = rowp.tile([1, 1], F32, tag="rsum", name=f"rsum_{b}")
            nc.vector.reciprocal(out=rsum, in_=sumexp)
            nc.vector.tensor_scalar_mul(sc_row, sc_row, rsum)
            nc.scalar.dma_start(out=att_sc[b:b + 1, :], in_=sc_row)
            bc = bcp.tile([P, l], BF16, tag="bc", name=f"bc_{b}")
            bc_src = bass.AP(tensor=row_dram.tensor, offset=row_dram.offset,
                             ap=[[0, P], [1, l]])
            if BC_ENGINE == "scalar":
                nc.scalar.dma_start(out=bc, in_=bc_src)
            else:
                nc.gpsimd.dma_start(out=bc, in_=bc_src)

            # ctx_unnorm[e] = sum_l exp_l * sem[b, l, e]: DVE mults (bf16 2x,
            # L-halves for finer WAR granularity on semT), then free-dim
            # reductions split between ACT (activation accum_out, a free
            # engine) and DVE (tensor_reduce, 1x) to balance load.
            LH = LT // 2
            ctxcol = ttrp.tile([P, EC], F32, tag="ctxcol", name=f"ctxcol_{b}")
            sdump = ttrp.tile([P, l], BF16, tag="sdump", name=f"sdump_{b}")
            scratch = ttrp.tile([P, 2, LT, P], BF16, tag="scratch",
                                name=f"scr_{b}")
            bc2 = bass.AP(tensor=bc.tensor, offset=bc.offset,
                          ap=[list(bc.ap[0]), [0, 2], [P, LH], [1, P]])
            bc2h = bass.AP(tensor=bc.tensor, offset=bc.offset + LH * P,
                           ap=[list(bc.ap[0]), [0, 2], [P, LH], [1, P]])
            for cp in range(EC // 2):
                for hf, bch in ((0, bc2), (1, bc2h)):
                    in0 = semT[:, hf * LH:(hf + 1) * LH,
                               2 * cp:2 * cp + 2, :].rearrange(
                        "p t c q -> p c t q")
                    nc.vector.tensor_mul(
                        scratch[:, :, hf * LH:(hf + 1) * LH, :], in0, bch)
                for ci in range(2):
                    c = 2 * cp + ci
                    flat = scratch[:, ci, :, :].rearrange("p t q -> p (t q)")
                    if ci == 0:
                        nc.scalar.activation(
                            out=sdump, in_=flat, func=AF.Identity,
                            bias=zero1b, scale=1.0,
                            accum_out=ctxcol[:, c:c + 1],
                        )
                    else:
                        nc.vector.tensor_reduce(
                            out=ctxcol[:, c:c + 1], in_=flat,
                            axis=mybir.AxisListType.X, op=ALU.add,
                        )
            nc.vector.tensor_copy(out=ctx_all[:, :, b], in_=ctxcol)

        # ---- att_out = ctx.T @ W_enc + b_enc, batched over blc ----
        ps_att = ps_enc.tile([blc, a], F32, tag="enc")
        for c in range(EC):
            nc.tensor.matmul(
                ps_att,
                lhsT=ctx_all[:, c, :],
                rhs=w_enc_sb[:, c, :],
                start=(c == 0),
                stop=(c == EC - 1),
            )
        se_all = consts.tile([blc, 1], F32)
        nc.sync.dma_start(out=se_all, in_=se_dram)
        rse_all = consts.tile([blc, 1], F32)
        nc.vector.reciprocal(out=rse_all, in_=se_all)
        att_final = consts.tile([blc, a], F32)
        nc.vector.tensor_scalar(att_final, ps_att, rse_all, None, op0=ALU.mult)
        nc.vector.tensor_add(att_final, att_final, b_enc_row)
        nc.sync.dma_start(out=att_out, in_=att_final)

    nc.compile()
    return nc


_CACHE = {}


def _get_nc(blc, l, e, a, h, num_devices):
    key = (blc, l, e, a, h, num_devices)
    if key not in _CACHE:
        _CACHE[key] = build_bass(blc, l, e, a, h, num_devices=num_devices)
    return _CACHE[key]


def make_in_maps(sem_enc_output, dec_hidden_state, W_enc, b_enc, W_dec, b_dec,
                 W_full, n_cores):
    B = sem_enc_output.shape[0]
    blc = B // n_cores
    bf = ml_dtypes.bfloat16
    w_enc_bf = np.ascontiguousarray(W_enc, np.float32).astype(bf)
    w_dec_bf = np.ascontiguousarray(W_dec, np.float32).astype(bf)
    wf_bf = np.ascontiguousarray(W_full[:, 0], np.float32).astype(bf)
    b_enc = np.ascontiguousarray(b_enc, np.float32)
    b_dec = np.ascontiguousarray(b_dec, np.float32)
    sem = np.ascontiguousarray(sem_enc_output, np.float32)
    dec = np.ascontiguousarray(dec_hidden_state, np.float32)
    in_maps = []
    for i in range(n_cores):
        sl = slice(i * blc, (i + 1) * blc)
        in_maps.append({
            "sem": sem[sl],
            "dect_bf": np.ascontiguousarray(dec[sl].T).astype(bf),
            "w_enc_bf": w_enc_bf,
            "w_dec_bf": w_dec_bf,
            "b_enc": b_enc,
            "b_dec": b_dec,
            "wf_bf": wf_bf,
        })
    return in_maps


def kernel(sem_enc_output, dec_hidden_state, W_enc, b_enc, W_dec, b_dec,
           W_full, b_full, _trace=False):
    B, L, E = sem_enc_output.shape
    H = dec_hidden_state.shape[1]
    A = W_enc.shape[1]
    n_cores = 8
    blc = B // n_cores

    nc = _get_nc(blc, L, E, A, H, n_cores)
    in_maps = make_in_maps(sem_enc_output, dec_hidden_state, W_enc, b_enc,
                           W_dec, b_dec, W_full, n_cores)
    res = bass_utils.run_bass_kernel_spmd(
        nc, in_maps, core_ids=list(range(n_cores)), trace=_trace,
    )
    att_out = np.concatenate([r["att_out"] for r in res.results], axis=0)
    att_sc = np.concatenate([r["att_sc"] for r in res.results], axis=0)
    kernel.last_results = res
    return att_out.astype(np.float32), att_sc.astype(np.float32)
